# revision 6
# baseline (speedup 1.0000x reference)
"""GAT attention layer (gnn_message_passing) on 8 TRN2 NeuronCores.

Math (reference):
    h = inp @ W;  s1 = h @ a1;  s2 = h @ a2
    e = leaky_relu(s1 + s2^T, 0.2);  scores = where(adj>0, e, -9e15)
    out = elu(softmax_row(scores) @ h)

Device algorithm (per core, rows R = N/8):
  Softmax without max-subtraction (scores are O(30), exp fits fp32 easily;
  masked entries contribute exactly 0 via P = adj * exp(e)):
      out[i,:] = elu( (sum_j P[i,j] h[j,:]) / (sum_j P[i,j]) )
  Everything is built in the TRANSPOSED orientation [j (partitions), i (free)]
  so the attention matmul needs no on-chip transposes:
      P^T[j,i] = adjT[j,i] * exp(lrelu(s1[i] + s2[j]))
  exp(lrelu(x)) with slope a<1 is max(exp(x), exp(a*x)) (exp is monotonic),
  so the leaky pass disappears into two biased Exp activations.
  - s2[j] (and a*s2[j]) are per-partition ACT biases (free from the h pass)
  - s1[i] is a small broadcast tile [128, R]
  - adjT is prepared host-side (layout-only transform of the input)
  num/denom in one matmul: rhs = [h | ones] (N = OUT+1), lhsT = P^T slices.

Host-side work is layout only: slicing/transposition of inputs. All FLOPs
(h, s1, s2, scores, softmax, attention@h, elu) happen on device.
"""
import sys

sys.path.insert(0, "/opt/trn_rl_repo")

import numpy as np

import concourse.bass as bass
import concourse.mybir as mybir
from concourse.tile import TileContext
from concourse.bass_utils import run_bass_kernel_spmd

F32 = mybir.dt.float32
I32 = mybir.dt.int32
AF = mybir.ActivationFunctionType
ALU = mybir.AluOpType

ALPHA = 0.2
N_CORES = 8


# ---------------------------------------------------------------------------
# walrus workaround: this build rejects >1 inline sync-wait per instruction
# ("Too many sync wait commands"); move the excess into same-engine NoOps.
# ---------------------------------------------------------------------------
def split_excess_waits(nc, nop_capacity=1):
    counter = 0
    for f in nc.m.functions:
        for bb in f.blocks:
            out = []
            changed = False
            for inst in bb.instructions:
                si = inst.sync_info
                max_inline = 0 if isinstance(inst, mybir.InstDrain) else 1
                if si is not None and len(si.on_wait) > max_inline:
                    waits = list(si.on_wait)
                    if max_inline:
                        excess, keep = waits[:-max_inline], waits[-max_inline:]
                    else:
                        excess, keep = waits, []
                    for s in range(0, len(excess), nop_capacity):
                        counter += 1
                        nop = mybir.InstNoOp(
                            name=f"WSPLIT-{counter}", ins=[], outs=[]
                        )
                        nop.engine = inst.engine
                        nop.sync_info = mybir.SyncInfo(
                            on_wait=excess[s:s + nop_capacity], on_update=[]
                        )
                        out.append(nop)
                    inst.sync_info = mybir.SyncInfo(
                        on_wait=keep, on_update=list(si.on_update)
                    )
                    changed = True
                out.append(inst)
            if changed:
                bb.instructions = out


# ---------------------------------------------------------------------------
# kernel builder (parameterised so a scaled-down version can run in CoreSim)
# ---------------------------------------------------------------------------
def build_nc(NJ, R, IN, OUT, acc_banks=6, ispan_adj_bufs=3, split_waits=True):
    P = 128
    KC = IN // P          # contraction chunks for inp @ W
    JC = NJ // P          # j (column/source-node) chunks
    OC = R // P           # own-row chunks
    G = -(-OC // acc_banks)   # row groups so live accumulators <= acc_banks
    OCG = OC // G
    assert OCG * G == OC
    ISPAN = OCG * P       # free width of the transposed score tiles
    CC = OUT // P         # chunks of the OUT dim (for W^T @ a12)
    E = OUT + 2           # h | s1 | s2 columns from the fused h pass
    HW_COLS = OUT + 3     # sbuf h tile: h | ones | s2 | ALPHA*s2

    nc = bass.Bass()
    adjT = nc.declare_dram_parameter("adjT", [NJ, R], I32, isOutput=False)
    inpT = nc.declare_dram_parameter("inpT", [IN, NJ], F32, isOutput=False)
    inpTo = nc.declare_dram_parameter("inpTo", [IN, R], F32, isOutput=False)
    W = nc.declare_dram_parameter("W", [IN, OUT], F32, isOutput=False)
    WT = nc.declare_dram_parameter("WT", [OUT, IN], F32, isOutput=False)
    a12 = nc.declare_dram_parameter("a12", [OUT, 2], F32, isOutput=False)
    ident = nc.declare_dram_parameter("ident", [P, P], F32, isOutput=False)
    out_own = nc.declare_dram_parameter("out", [R, OUT], F32, isOutput=True)
    s1_dram = nc.dram_tensor("s1_scratch", [R], F32)

    with TileContext(nc) as tc:
        with (
            tc.tile_pool(name="const", bufs=1) as constp,
            tc.tile_pool(name="wts", bufs=1) as wts,
            tc.tile_pool(name="hpool", bufs=1) as hpool,
            tc.tile_pool(name="s1p", bufs=1) as s1p,
            tc.tile_pool(name="inp_t", bufs=8) as inp_p,
            tc.tile_pool(name="adjp", bufs=ispan_adj_bufs) as adjp,
            tc.tile_pool(name="workp", bufs=2) as workp,
            tc.tile_pool(name="ep", bufs=4) as ep,
            tc.tile_pool(name="pmisc", bufs=2, space="PSUM") as pmisc,
            tc.tile_pool(name="pacc", bufs=1, space="PSUM") as pacc,
        ):
            # ---- stage 0: weights ----
            ident_sb = constp.tile([P, P], F32, tag="ident")
            nc.sync.dma_start(out=ident_sb[:, :], in_=ident[:, :])
            wt_sb = []
            a12_sb = []
            for cc in range(CC):
                t = wts.tile([P, IN], F32, tag=f"wt{cc}", name=f"wt{cc}")
                nc.sync.dma_start(out=t[:, :], in_=WT[cc * P:(cc + 1) * P, :])
                wt_sb.append(t)
                t2 = wts.tile([P, 2], F32, tag=f"a12_{cc}", name=f"a12_{cc}")
                nc.sync.dma_start(out=t2[:, :], in_=a12[cc * P:(cc + 1) * P, :])
                a12_sb.append(t2)
            rhs_ext = []
            for kc in range(KC):
                t = wts.tile([P, E], F32, tag=f"rhsext{kc}", name=f"rhsext{kc}")
                nc.sync.dma_start(
                    out=t[:, 0:OUT], in_=W[kc * P:(kc + 1) * P, :]
                )
                rhs_ext.append(t)
            # w12[k, 0:2] = (W @ [a1 a2])[k]  via  WT-blocks^T @ a12-blocks
            for kc in range(KC):
                pw = pmisc.tile([P, 2], F32, tag="pm")
                for cc in range(CC):
                    nc.tensor.matmul(
                        pw[:, :],
                        wt_sb[cc][:, kc * P:(kc + 1) * P],
                        a12_sb[cc][:, :],
                        start=(cc == 0),
                        stop=(cc == CC - 1),
                    )
                nc.vector.tensor_copy(rhs_ext[kc][:, OUT:OUT + 2], pw[:, :])

            # ---- stage 1b: s1 of own rows, then broadcast tile ----
            s1_stage = s1p.tile([P, OC], F32, tag="s1stage")
            for oc in range(OC):
                its = []
                for kc in range(KC):
                    t = inp_p.tile([P, P], F32, tag="ito", name="ito")
                    nc.sync.dma_start(
                        out=t[:, :],
                        in_=inpTo[kc * P:(kc + 1) * P, oc * P:(oc + 1) * P],
                    )
                    its.append(t)
                ps1 = pmisc.tile([P, 1], F32, tag="pm")
                for kc in range(KC):
                    nc.tensor.matmul(
                        ps1[:, :],
                        its[kc][:, :],
                        rhs_ext[kc][:, OUT:OUT + 1],
                        start=(kc == 0),
                        stop=(kc == KC - 1),
                    )
                nc.vector.tensor_copy(s1_stage[:, oc:oc + 1], ps1[:, :])
            # transpose [128, OC] -> [OC, 128], flatten to DRAM, re-read
            # broadcast across partitions.
            pt1 = pmisc.tile([P, P], F32, tag="pm")
            nc.tensor.matmul(
                pt1[:OC, :], s1_stage[:, :], ident_sb[:, :], is_transpose=True
            )
            s1rows = s1p.tile([P, P], F32, tag="s1rows")
            nc.vector.tensor_copy(s1rows[:OC, :], pt1[:OC, :])
            nc.sync.dma_start(
                out=s1_dram[:].rearrange("(a b) -> a b", b=P),
                in_=s1rows[:OC, :],
            )
            s1bc = s1p.tile([P, R], F32, tag="s1bc")
            nc.sync.dma_start(
                out=s1bc[:, :], in_=s1_dram[:].partition_broadcast(P)
            )

            # ---- stage 1: fused h | s1 | s2 for all NJ rows ----
            h_sb = []
            for jc in range(JC):
                its = []
                for kc in range(KC):
                    t = inp_p.tile([P, P], F32, tag="it", name="it")
                    nc.sync.dma_start(
                        out=t[:, :],
                        in_=inpT[kc * P:(kc + 1) * P, jc * P:(jc + 1) * P],
                    )
                    its.append(t)
                ph = pmisc.tile([P, E], F32, tag="pm")
                for kc in range(KC):
                    nc.tensor.matmul(
                        ph[:, :],
                        its[kc][:, :],
                        rhs_ext[kc][:, :],
                        start=(kc == 0),
                        stop=(kc == KC - 1),
                    )
                h = hpool.tile([P, HW_COLS], F32, tag=f"h{jc}", name=f"h{jc}")
                nc.vector.tensor_copy(h[:, 0:OUT], ph[:, 0:OUT])
                nc.vector.memset(h[:, OUT:OUT + 1], 1.0)
                # s2 and ALPHA*s2 columns: per-partition Exp biases later
                nc.vector.tensor_copy(h[:, OUT + 1:OUT + 2], ph[:, OUT + 1:OUT + 2])
                nc.vector.tensor_scalar_mul(
                    h[:, OUT + 2:OUT + 3], ph[:, OUT + 1:OUT + 2], ALPHA
                )
                h_sb.append(h)

            # ---- stage 2: masked-exp scores (transposed) + attention matmul ----
            for g in range(G):
                accs = [
                    pacc.tile([P, OUT + 1], F32, tag=f"acc{m}", name=f"acc{m}")
                    for m in range(OCG)
                ]
                for jc in range(JC):
                    at = adjp.tile([P, ISPAN], I32, tag="at")
                    nc.sync.dma_start(
                        out=at[:, :],
                        in_=adjT[
                            jc * P:(jc + 1) * P,
                            g * ISPAN:(g + 1) * ISPAN,
                        ],
                    )
                    e1 = workp.tile([P, ISPAN], F32, tag="e1")
                    nc.scalar.activation(
                        e1[:, :],
                        s1bc[:, g * ISPAN:(g + 1) * ISPAN],
                        AF.Exp,
                        bias=h_sb[jc][:, OUT + 1:OUT + 2],
                        scale=1.0,
                    )
                    e2 = workp.tile([P, ISPAN], F32, tag="e2")
                    nc.scalar.activation(
                        e2[:, :],
                        s1bc[:, g * ISPAN:(g + 1) * ISPAN],
                        AF.Exp,
                        bias=h_sb[jc][:, OUT + 2:OUT + 3],
                        scale=ALPHA,
                    )
                    mx = workp.tile([P, ISPAN], F32, tag="mx")
                    nc.vector.tensor_tensor(
                        mx[:, :], e1[:, :], e2[:, :], ALU.max
                    )
                    pT = workp.tile([P, ISPAN], F32, tag="pT")
                    nc.vector.tensor_tensor(
                        pT[:, :], mx[:, :], at[:, :], ALU.mult
                    )
                    for m in range(OCG):
                        nc.tensor.matmul(
                            accs[m][:, :],
                            pT[:, m * P:(m + 1) * P],
                            h_sb[jc][:, 0:OUT + 1],
                            start=(jc == 0),
                            stop=(jc == JC - 1),
                        )
                # epilogue: normalise + elu + store
                for m in range(OCG):
                    acc = accs[m]
                    r = ep.tile([P, 1], F32, tag="r")
                    nc.vector.reciprocal(r[:, :], acc[:, OUT:OUT + 1])
                    t = ep.tile([P, OUT], F32, tag="t")
                    nc.vector.tensor_scalar_mul(t[:, :], acc[:, 0:OUT], r[:, :])
                    ng = ep.tile([P, OUT], F32, tag="ng")
                    nc.vector.tensor_scalar_min(ng[:, :], t[:, :], 0.0)
                    en = ep.tile([P, OUT], F32, tag="en")
                    nc.scalar.activation(en[:, :], ng[:, :], AF.Exp)
                    ps = ep.tile([P, OUT], F32, tag="ps")
                    nc.vector.tensor_scalar_max(ps[:, :], t[:, :], 0.0)
                    res = ep.tile([P, OUT], F32, tag="res")
                    # elu = max(x,0) + (exp(min(x,0)) - 1)
                    nc.vector.scalar_tensor_tensor(
                        res[:, :], ps[:, :], -1.0, en[:, :],
                        op0=ALU.add, op1=ALU.add,
                    )
                    row = (g * OCG + m) * P
                    nc.sync.dma_start(
                        out=out_own[row:row + P, :], in_=res[:, :]
                    )

    if split_waits:  # CoreSim chokes on the NoOps; only needed for walrus
        split_excess_waits(nc)
    return nc


# ---------------------------------------------------------------------------
# host wrapper
# ---------------------------------------------------------------------------
_CACHE = {}


def _get_nc(NJ, R, IN, OUT):
    key = (NJ, R, IN, OUT)
    if key not in _CACHE:
        _CACHE[key] = build_nc(NJ, R, IN, OUT)
    return _CACHE[key]


def prep_in_maps(inp, adj, W, a1, a2, n_cores=N_CORES):
    """Host-side layout prep: slicing + transposition only."""
    N, IN = inp.shape
    OUT = W.shape[1]
    R = N // n_cores
    inpT = np.ascontiguousarray(inp.T)
    WT = np.ascontiguousarray(W.T)
    a12 = np.ascontiguousarray(np.concatenate([a1, a2], axis=1))
    ident = np.eye(128, dtype=np.float32)
    in_maps = []
    for c in range(n_cores):
        sl = slice(c * R, (c + 1) * R)
        in_maps.append({
            "adjT": np.ascontiguousarray(adj[sl, :].T),
            "inpT": inpT,
            "inpTo": np.ascontiguousarray(inp[sl, :].T),
            "W": W,
            "WT": WT,
            "a12": a12,
            "ident": ident,
        })
    return in_maps, R, IN, OUT


def kernel(inp, adj, W, a1, a2):
    inp = np.asarray(inp, dtype=np.float32)
    adj = np.asarray(adj, dtype=np.int32)
    W = np.asarray(W, dtype=np.float32)
    a1 = np.asarray(a1, dtype=np.float32)
    a2 = np.asarray(a2, dtype=np.float32)
    N = inp.shape[0]
    in_maps, R, IN, OUT = prep_in_maps(inp, adj, W, a1, a2)
    nc = _get_nc(N, R, IN, OUT)
    res = run_bass_kernel_spmd(nc, in_maps, list(range(N_CORES)))
    return np.concatenate(
        [res.results[c]["out"] for c in range(N_CORES)], axis=0
    )
